# revision 12
# baseline (speedup 1.0000x reference)
"""Trainium2 Bass kernel for the LocalGNOBlock (windowed GNN message passing).

Math restructuring (vs the naive 12x full MLP evaluations):
  msg first layer is linear over concat([h_i, h_j, dc]):
      z_d[i] = (A - C)[i] + (B + C)[i+d] + b1,  d in {+-1..+-6}
  where A = h @ W1a, B = h @ W1b, C = coord x w1c (rank-1).
  The second msg layer is summed over edges BEFORE the matmul:
      agg_pre = (sum_d silu(z_d)) @ W2
  Aggregate divide-by-count folds into W2 (interior count == 12) with a
  6-column fixup at each sequence end.  LayerNorm stats are computed with
  ones-vector matmuls (channel dim lives on partitions).

Pass 2 (normalize) is matmul-free: mu/rstd rows are round-tripped through
DRAM and broadcast to 128 partitions with stride-0 DMA; the normalize is
two fp16 2x-mode tensor_tensor ops plus one 4x-mode tensor_scalar with
per-partition (g, b) operands.  Output is written fp16 and cast on host.

Sharding: batch dim B=8 -> one batch element per NeuronCore (no halo needed).
"""

import numpy as np

K = 6
HID = 128
N = 16384
B = 8
EPS = 1e-5
T = 512                 # token chunk (matmul + elementwise granularity)
NCH = N // T            # 32 chunks
OFF0 = 8                # D_full column of token 0 (even, for fp16 alignment)
NCOL = N + 2 * OFF0     # D_full width

# offsets: even offsets read D_A, odd offsets read D_B (shifted copy)
NEG_EVEN = [-6, -4, -2]
ODD = [-5, -3, -1, 1, 3, 5]
POS_EVEN = [2, 4, 6]
SEG_ORDER = NEG_EVEN + ODD + POS_EVEN  # 12 segments in Z

_compiled = None


def _build_bass(dt_act):
    import concourse.bacc as bacc
    import concourse.bass as bass
    import concourse.tile as tile
    from concourse import mybir

    f32 = mybir.dt.float32
    DT = dt_act

    nc = bacc.Bacc("TRN2", target_bir_lowering=False, debug=False)

    # ---- DRAM I/O ----
    hT = nc.dram_tensor("hT", [HID, N], DT, kind="ExternalInput")
    coordR = nc.dram_tensor("coordR", [1, N], DT, kind="ExternalInput")
    W1a = nc.dram_tensor("W1a", [HID, HID], DT, kind="ExternalInput")
    W1b = nc.dram_tensor("W1b", [HID, HID], DT, kind="ExternalInput")
    w1c = nc.dram_tensor("w1c", [1, HID], DT, kind="ExternalInput")      # +w1c
    w1cn = nc.dram_tensor("w1cn", [1, HID], DT, kind="ExternalInput")    # -w1c
    W2s = nc.dram_tensor("W2s", [HID, HID], DT, kind="ExternalInput")     # W2/12
    U1a = nc.dram_tensor("U1a", [HID, HID], DT, kind="ExternalInput")
    U1b = nc.dram_tensor("U1b", [HID, HID], DT, kind="ExternalInput")
    U2 = nc.dram_tensor("U2", [HID, HID], DT, kind="ExternalInput")
    b1c = nc.dram_tensor("b1c", [HID, 1], f32, kind="ExternalInput")      # msg_b1
    buc = nc.dram_tensor("buc", [HID, 1], f32, kind="ExternalInput")      # upd_b1 + b2@U1b
    b2u = nc.dram_tensor("b2u", [1, HID], DT, kind="ExternalInput")      # upd_b2 row
    g_col = nc.dram_tensor("g_col", [HID, 1], f32, kind="ExternalInput")  # ln_g
    b_col = nc.dram_tensor("b_col", [HID, 1], f32, kind="ExternalInput")  # ln_b
    ident = nc.dram_tensor("ident", [HID, HID], DT, kind="ExternalInput")
    fixf = nc.dram_tensor("fixf", [1, K], f32, kind="ExternalInput")      # 12/count head
    fixl = nc.dram_tensor("fixl", [1, K], f32, kind="ExternalInput")      # 12/count tail
    # band-select matrix: hot column = 1/128 (stats row packing)
    selb = nc.dram_tensor("selb", [HID, 2 * 2 * NCH - 1], DT, kind="ExternalInput")
    outT = nc.dram_tensor("outT", [HID, N], DT, kind="ExternalOutput")

    Silu = mybir.ActivationFunctionType.Silu
    Sqrt = mybir.ActivationFunctionType.Sqrt

    with tile.TileContext(nc) as tc:
        with (
            tc.tile_pool(name="singles", bufs=1) as singles,
            tc.tile_pool(name="big", bufs=1) as big,
            tc.tile_pool(name="work", bufs=1) as work,
            tc.tile_pool(name="zpool", bufs=3) as zpool,
            tc.tile_pool(name="opool", bufs=3) as opool,
            tc.tile_pool(name="dramp", bufs=1, space="DRAM") as dramp,
            tc.tile_pool(name="psA", bufs=1, space="PSUM") as psA,
            tc.tile_pool(name="psB", bufs=1, space="PSUM") as psB,
            tc.tile_pool(name="psS", bufs=1, space="PSUM") as psS,
        ):
            # ---- constants into SBUF (spread across DMA queues) ----
            sW1a = singles.tile([HID, HID], DT)
            sW1b = singles.tile([HID, HID], DT)
            sW2s = singles.tile([HID, HID], DT)
            sU1a = singles.tile([HID, HID], DT)
            sU1b = singles.tile([HID, HID], DT)
            sU2 = singles.tile([HID, HID], DT)
            sIdent = singles.tile([HID, HID], DT)
            sw1c = singles.tile([1, HID], DT)
            sw1cn = singles.tile([1, HID], DT)
            sb2u = singles.tile([1, HID], DT)
            sb1 = singles.tile([HID, 1], f32)
            sbu = singles.tile([HID, 1], f32)
            sg = singles.tile([HID, 1], f32)
            sb = singles.tile([HID, 1], f32)
            ssel = singles.tile([HID, 2 * 2 * NCH - 1], DT)
            # phase-a-critical weights first on the scalar/gpsimd queues; the
            # sync queue starts with the first h/coord chunks (issued in
            # load_chunk below, which the scheduler hoists ahead of these).
            qs = [nc.scalar, nc.gpsimd, nc.sync]
            loads = [(sW1b, W1b), (sw1c, w1c), (sW1a, W1a), (sw1cn, w1cn),
                     (sb1, b1c), (sW2s, W2s), (sU1a, U1a), (sU1b, U1b),
                     (sU2, U2), (sIdent, ident), (sb2u, b2u), (sbu, buc),
                     (sg, g_col), (sb, b_col), (ssel, selb)]
            for i, (sbuf_t, dr) in enumerate(loads):
                qs[i % 2].dma_start(out=sbuf_t, in_=dr[:, :])
            # broadcast [1,6] -> [128,6] fix tiles
            sfixf = singles.tile([HID, K], f32)
            sfixl = singles.tile([HID, K], f32)

            def bcast_rows(a, p=HID):
                return bass.AP(tensor=a.tensor, offset=a.offset,
                               ap=[[0, p]] + list(a.ap[1:]))

            nc.gpsimd.dma_start(out=sfixf, in_=bcast_rows(fixf[0:1, :]))
            nc.gpsimd.dma_start(out=sfixl, in_=bcast_rows(fixl[0:1, :]))
            sones_row = singles.tile([1, T], DT)
            nc.vector.memset(sones_row, 1.0)
            seps = singles.tile([NCH, 1], f32)
            nc.vector.memset(seps, float(EPS))

            # ---- big persistent buffers ----
            D_A = big.tile([HID, NCOL], DT)      # token j at col OFF0 + j
            D_B = big.tile([HID, NCOL], DT)      # token j at col OFF0 + 1 + j
            x_full = big.tile([HID, N], DT)
            # zero halo columns of D so boundary silu stays finite
            nc.vector.memset(D_A[:, 0:OFF0], 0.0)
            nc.vector.memset(D_A[:, OFF0 + N:NCOL], 0.0)
            nc.vector.memset(D_B[:, 0:OFF0 + 1], 0.0)
            nc.vector.memset(D_B[:, OFF0 + 1 + N:NCOL], 0.0)

            # LN stats: rows [0:32] = E[x]/chunk, [32:64] = E[x^2]/chunk
            st_ps = psS.tile([2 * NCH, T], f32)

            hts = {}
            crd = {}

            def load_chunk(c):
                ht = work.tile([HID, T], DT, tag="ht", bufs=6)
                nc.sync.dma_start(out=ht, in_=hT[:, c * T:(c + 1) * T])
                co = work.tile([1, T], DT, tag="co", bufs=6)
                nc.sync.dma_start(out=co, in_=coordR[:, c * T:(c + 1) * T])
                hts[c] = ht
                crd[c] = co

            def phase_a(c):
                # D chunk = W1b.T @ h  +  w1c x coord   (PSUM accumulate)
                d_ps = psA.tile([HID, T], f32, tag="de", bufs=2)
                nc.tensor.matmul(d_ps, sW1b, hts[c], start=True, stop=False)
                nc.tensor.matmul(d_ps, sw1c, crd[c], start=False, stop=True)
                col = OFF0 + c * T
                nc.vector.tensor_copy(D_A[:, col:col + T], d_ps)
                nc.gpsimd.tensor_copy(
                    out=D_B[:, col + 1:col + 1 + T], in_=D_A[:, col:col + T])

            def seg_in1(tile_ap, col, nseg):
                # [128, nseg, T] AP over D with outer column-stride 2
                s = tile_ap[:, col:col + T]
                return bass.AP(tensor=s.tensor, offset=s.offset,
                               ap=[s.ap[0], [2, nseg], [1, T]])

            def phase_b(t):
                ht, co = hts[t], crd[t]
                # E chunk = W1a.T @ h - w1c x coord
                e_ps = psA.tile([HID, T], f32, tag="de", bufs=2)
                nc.tensor.matmul(e_ps, sW1a, ht, start=True, stop=False)
                nc.tensor.matmul(e_ps, sw1cn, co, start=False, stop=True)
                e_sb = work.tile([HID, T], DT, tag="esb", bufs=3)
                nc.vector.tensor_copy(e_sb, e_ps)

                # Z: 12 segments of E + shifted D, 3 stride-2 groups
                z = zpool.tile([HID, 12 * T], DT, tag="z")
                zv = z.rearrange("p (s t) -> p s t", t=T)

                def e_bc(ap_dims):
                    return bass.AP(tensor=e_sb.tensor, offset=e_sb.offset,
                                   ap=[e_sb.ap[0]] + ap_dims + [[1, T]])
                base = t * T
                # even offsets: two stride-2 triples (-6,-4,-2) and (+2,+4,+6)
                # = 4D AP over D_A with a group-jump of 8 columns; dst segments
                # (0,1,2) and (9,10,11) with a group-jump of 9*T.
                sA = D_A[:, OFF0 + base + NEG_EVEN[0]:]
                zva = zv[:, 0:1, :]
                nc.vector.tensor_tensor(
                    out=bass.AP(tensor=zva.tensor, offset=zva.offset,
                                ap=[zva.ap[0], [9 * T, 2], [T, 3], [1, T]]),
                    in0=e_bc([[0, 2], [0, 3]]),
                    in1=bass.AP(tensor=sA.tensor, offset=sA.offset,
                                ap=[sA.ap[0], [8, 2], [2, 3], [1, T]]),
                    op=mybir.AluOpType.add)
                # odd offsets: one stride-2 sextet (-5,-3,-1,+1,+3,+5) on D_B
                nc.vector.tensor_tensor(
                    out=zv[:, 3:9, :],
                    in0=e_bc([[0, 6]]),
                    in1=seg_in1(D_B, OFF0 + 1 + base + ODD[0], 6),
                    op=mybir.AluOpType.add)

                # silu over all 12 segments at once (bias = msg_b1)
                nc.scalar.activation(z, z, Silu, bias=sb1, scale=1.0)

                # zero invalid boundary columns (torn edges of the sequence)
                if t == 0:
                    for s, d in enumerate(SEG_ORDER):
                        if d < 0:
                            nc.vector.memset(zv[:, s, 0:-d], 0.0)
                if t == NCH - 1:
                    for s, d in enumerate(SEG_ORDER):
                        if d > 0:
                            nc.vector.memset(zv[:, s, T - d:T], 0.0)

                # agg_pre = sum_s silu(z_s) @ W2s   (PSUM accumulation)
                a_ps = psB.tile([HID, T], f32, tag="agg", bufs=2)
                for s in range(12):
                    nc.tensor.matmul(a_ps, sW2s, zv[:, s, :],
                                     start=(s == 0), stop=(s == 11))
                agg = work.tile([HID, T], DT, tag="agg_sb", bufs=3)
                nc.vector.tensor_copy(agg, a_ps)
                if t == 0:
                    nc.vector.tensor_tensor(out=agg[:, 0:K], in0=a_ps[:, 0:K],
                                            in1=sfixf, op=mybir.AluOpType.mult)
                if t == NCH - 1:
                    nc.vector.tensor_tensor(out=agg[:, T - K:T],
                                            in0=a_ps[:, T - K:T],
                                            in1=sfixl, op=mybir.AluOpType.mult)

                # update MLP
                u_ps = psA.tile([HID, T], f32, tag="upd", bufs=2)
                nc.tensor.matmul(u_ps, sU1a, ht, start=True, stop=False)
                nc.tensor.matmul(u_ps, sU1b, agg, start=False, stop=True)
                s2 = work.tile([HID, T], DT, tag="s2", bufs=3)
                nc.scalar.activation(s2, u_ps, Silu, bias=sbu, scale=1.0)

                # x = h + silu@U2 + b2u  (all accumulated in PSUM)
                x_ps = psA.tile([HID, T], f32, tag="xps", bufs=1)
                nc.tensor.matmul(x_ps, sU2, s2, start=True, stop=False)
                nc.tensor.matmul(x_ps, sb2u, sones_row, start=False, stop=False)
                nc.tensor.matmul(x_ps, sIdent, ht, start=False, stop=True)
                x_sb = x_full[:, base:base + T]
                nc.vector.tensor_copy(x_sb, x_ps)
                x2 = work.tile([HID, T], DT, tag="x2", bufs=2)
                nc.gpsimd.tensor_mul(x2, x_sb, x_sb)
                # LN stats rows: band-select lhsT packs E[x] into psum row t
                # and E[x^2] into row NCH+t of one accumulating [64,T] bank
                hot = 2 * NCH - 1
                nc.tensor.matmul(st_ps[:, :], ssel[:, hot - t:hot - t + 2 * NCH],
                                 x_sb, start=(t == 0), stop=False)
                nc.tensor.matmul(st_ps[:, :],
                                 ssel[:, hot - NCH - t:hot - t + NCH],
                                 x2, start=False, stop=(t == NCH - 1))

            # ---------------- pass 1 ----------------
            load_chunk(0)
            for c in range(NCH + 1):
                if c < NCH:
                    if c + 1 < NCH:
                        load_chunk(c + 1)
                    phase_a(c)
                if c >= 1:
                    phase_b(c - 1)

            # ---------------- LN stats math ----------------
            # mu path first so the mu broadcast chain starts early
            mu16 = work.tile([NCH, T], DT, tag="mu16")
            nc.vector.tensor_copy(mu16, st_ps[0:NCH, :])
            mu_dr = dramp.tile([NCH, T], DT)
            nc.sync.dma_start(out=mu_dr, in_=mu16)
            ex32 = work.tile([NCH, T], f32, tag="ex")
            nc.vector.tensor_copy(ex32, st_ps[0:NCH, :])
            musq = work.tile([NCH, T], f32, tag="musq")
            nc.vector.tensor_tensor(out=musq, in0=ex32, in1=ex32,
                                    op=mybir.AluOpType.mult)
            var = work.tile([NCH, T], f32, tag="var")
            nc.vector.tensor_tensor(out=var, in0=st_ps[NCH:2 * NCH, :], in1=musq,
                                    op=mybir.AluOpType.subtract)
            sd32 = work.tile([NCH, T], f32, tag="sd32")
            nc.scalar.activation(sd32, var, Sqrt, bias=seps, scale=1.0)
            r32 = work.tile([NCH, T], f32, tag="r32")
            with nc.allow_low_precision(reason="rstd rows feed fp16 normalize"):
                nc.vector.reciprocal_approx_fast(out=r32, in_=sd32)
            r16 = work.tile([NCH, T], DT, tag="r16")
            nc.vector.tensor_copy(r16, r32)
            r_dr = dramp.tile([NCH, T], DT)
            nc.gpsimd.dma_start(out=r_dr, in_=r16)

            # ---------------- pass 2: normalize (no matmuls) ----------------
            Copy = mybir.ActivationFunctionType.Identity
            for t in range(NCH):
                base = t * T
                mu_bc = opool.tile([HID, T], DT, tag="mu_bc", bufs=6)
                r_bc = opool.tile([HID, T], DT, tag="r_bc", bufs=6)
                qa, qb = (nc.scalar, nc.gpsimd) if t % 2 == 0 else (nc.gpsimd, nc.scalar)
                qa.dma_start(out=mu_bc, in_=bcast_rows(mu_dr[t:t + 1, :]))
                qb.dma_start(out=r_bc, in_=bcast_rows(r_dr[t:t + 1, :]))
                t0 = work.tile([HID, T], DT, tag="t0", bufs=4)
                nc.vector.tensor_tensor(out=t0, in0=x_full[:, base:base + T],
                                        in1=mu_bc, op=mybir.AluOpType.subtract)
                t1 = work.tile([HID, T], DT, tag="t1", bufs=4)
                nc.vector.scalar_tensor_tensor(out=t1, in0=t0, scalar=sg,
                                               in1=r_bc,
                                               op0=mybir.AluOpType.mult,
                                               op1=mybir.AluOpType.mult)
                o = opool.tile([HID, T], DT, tag="o", bufs=6)
                nc.scalar.activation(o, t1, Copy, bias=sb, scale=1.0)
                nc.sync.dma_start(out=outT[:, base:base + T], in_=o)

    nc.compile()
    return nc


def _get_compiled(dt_name):
    global _compiled
    if _compiled is None:
        from concourse import mybir
        dt = {"bf16": mybir.dt.bfloat16, "fp16": mybir.dt.float16, "fp32": mybir.dt.float32}[dt_name]
        _compiled = _build_bass(dt)
    return _compiled


DT_NAME = "fp16"


def _sel_band(act_np):
    hot = 2 * NCH - 1
    sel = np.zeros((HID, 2 * 2 * NCH - 1), dtype=np.float32)
    sel[:, hot] = 1.0 / HID
    return sel.astype(act_np)


def kernel(**inputs):
    from concourse.bass_utils import run_bass_kernel_spmd

    h = np.asarray(inputs["h"], dtype=np.float32)
    coord = np.asarray(inputs["coord"], dtype=np.float32)
    msg_w1 = np.asarray(inputs["msg_w1"], dtype=np.float32)
    msg_b1 = np.asarray(inputs["msg_b1"], dtype=np.float32)
    msg_w2 = np.asarray(inputs["msg_w2"], dtype=np.float32)
    msg_b2 = np.asarray(inputs["msg_b2"], dtype=np.float32)
    upd_w1 = np.asarray(inputs["upd_w1"], dtype=np.float32)
    upd_b1 = np.asarray(inputs["upd_b1"], dtype=np.float32)
    upd_w2 = np.asarray(inputs["upd_w2"], dtype=np.float32)
    upd_b2 = np.asarray(inputs["upd_b2"], dtype=np.float32)
    ln_g = np.asarray(inputs["ln_g"], dtype=np.float32)
    ln_b = np.asarray(inputs["ln_b"], dtype=np.float32)

    import ml_dtypes
    act_np = {"bf16": ml_dtypes.bfloat16, "fp16": np.float16, "fp32": np.float32}[DT_NAME]

    W1a = msg_w1[:HID]
    W1b = msg_w1[HID:2 * HID]
    w1c = msg_w1[2 * HID]
    bias_u = upd_b1 + msg_b2 @ upd_w1[HID:2 * HID]
    W2s = msg_w2 / (2.0 * K)

    idx = np.arange(N)
    count = (np.minimum(idx, K) + np.minimum(N - 1 - idx, K)).astype(np.float32)
    fix = (2.0 * K) / count
    fixf = fix[:K].reshape(1, K).astype(np.float32)
    fixl = fix[N - K:].reshape(1, K).astype(np.float32)

    const = {
        "W1a": np.ascontiguousarray(W1a, dtype=act_np),
        "W1b": np.ascontiguousarray(W1b, dtype=act_np),
        "w1c": np.ascontiguousarray(w1c.reshape(1, HID), dtype=act_np),
        "w1cn": np.ascontiguousarray(-w1c.reshape(1, HID), dtype=act_np),
        "W2s": np.ascontiguousarray(W2s, dtype=act_np),
        "U1a": np.ascontiguousarray(upd_w1[:HID], dtype=act_np),
        "U1b": np.ascontiguousarray(upd_w1[HID:], dtype=act_np),
        "U2": np.ascontiguousarray(upd_w2, dtype=act_np),
        "b1c": np.ascontiguousarray(msg_b1.reshape(HID, 1), dtype=np.float32),
        "buc": np.ascontiguousarray(bias_u.reshape(HID, 1), dtype=np.float32),
        "b2u": np.ascontiguousarray(upd_b2.reshape(1, HID), dtype=act_np),
        "g_col": np.ascontiguousarray(ln_g.reshape(HID, 1), dtype=np.float32),
        "b_col": np.ascontiguousarray(ln_b.reshape(HID, 1), dtype=np.float32),
        "ident": np.ascontiguousarray(np.eye(HID), dtype=act_np),
        "fixf": fixf,
        "fixl": fixl,
        "selb": _sel_band(act_np),
    }

    in_maps = []
    for b in range(B):
        m = dict(const)
        m["hT"] = np.ascontiguousarray(h[b].T, dtype=act_np)
        m["coordR"] = np.ascontiguousarray(coord[b].reshape(1, N), dtype=act_np)
        in_maps.append(m)

    nc = _get_compiled(DT_NAME)
    res = run_bass_kernel_spmd(nc, in_maps, core_ids=list(range(B)))
    global LAST_RESULTS
    LAST_RESULTS = res
    out = np.stack([np.asarray(res.results[b]["outT"], dtype=np.float32).T
                    for b in range(B)])
    return np.ascontiguousarray(out)


# revision 15
# speedup vs baseline: 1.1515x; 1.1515x over previous
"""Trainium2 Bass kernel for the LocalGNOBlock (windowed GNN message passing).

Math restructuring (vs the naive 12x full MLP evaluations):
  msg first layer is linear over concat([h_i, h_j, dc]):
      z_d[i] = (A - C)[i] + (B + C)[i+d] + b1,  d in {+-1..+-6}
  where A = h @ W1a, B = h @ W1b, C = coord x w1c (rank-1).
  The second msg layer is summed over edges BEFORE the matmul:
      agg_pre = (sum_d silu(z_d)) @ W2
  Aggregate divide-by-count folds into W2 (interior count == 12) with a
  6-column fixup at each sequence end.  LayerNorm stats are computed with
  ones-vector matmuls (channel dim lives on partitions).

Pass 2 (normalize) is matmul-free: mu/rstd rows are round-tripped through
DRAM and broadcast to 128 partitions with stride-0 DMA; the normalize is
two fp16 2x-mode tensor_tensor ops plus one 4x-mode tensor_scalar with
per-partition (g, b) operands.  Output is written fp16 and cast on host.

Sharding: batch dim B=8 -> one batch element per NeuronCore (no halo needed).
"""

import numpy as np

K = 6
HID = 128
N = 16384
B = 8
EPS = 1e-5
T = 512                 # token chunk (matmul + elementwise granularity)
NCH = N // T            # 32 chunks
OFF0 = 8                # D_full column of token 0 (even, for fp16 alignment)
NCOL = N + 2 * OFF0     # D_full width

# offsets: even offsets read D_A, odd offsets read D_B (shifted copy)
NEG_EVEN = [-6, -4, -2]
ODD = [-5, -3, -1, 1, 3, 5]
POS_EVEN = [2, 4, 6]
SEG_ORDER = NEG_EVEN + ODD + POS_EVEN  # 12 segments in Z

_compiled = None


def _build_bass(dt_act):
    import concourse.bacc as bacc
    import concourse.bass as bass
    import concourse.tile as tile
    from concourse import mybir

    f32 = mybir.dt.float32
    DT = dt_act

    nc = bacc.Bacc("TRN2", target_bir_lowering=False, debug=False)

    # ---- DRAM I/O ----
    hT = nc.dram_tensor("hT", [HID, N], DT, kind="ExternalInput")
    coordR = nc.dram_tensor("coordR", [1, N], DT, kind="ExternalInput")
    W1a = nc.dram_tensor("W1a", [HID, HID], DT, kind="ExternalInput")
    W1b = nc.dram_tensor("W1b", [HID, HID], DT, kind="ExternalInput")
    w1c = nc.dram_tensor("w1c", [1, HID], DT, kind="ExternalInput")      # +w1c
    w1cn = nc.dram_tensor("w1cn", [1, HID], DT, kind="ExternalInput")    # -w1c
    W2s = nc.dram_tensor("W2s", [HID, HID], DT, kind="ExternalInput")     # W2/12
    U1a = nc.dram_tensor("U1a", [HID, HID], DT, kind="ExternalInput")
    U1b = nc.dram_tensor("U1b", [HID, HID], DT, kind="ExternalInput")
    U2 = nc.dram_tensor("U2", [HID, HID], DT, kind="ExternalInput")
    b1c = nc.dram_tensor("b1c", [HID, 1], f32, kind="ExternalInput")      # msg_b1
    buc = nc.dram_tensor("buc", [HID, 1], f32, kind="ExternalInput")      # upd_b1 + b2@U1b
    b2u = nc.dram_tensor("b2u", [1, HID], DT, kind="ExternalInput")      # upd_b2 row
    g_col = nc.dram_tensor("g_col", [HID, 1], f32, kind="ExternalInput")  # ln_g
    b_col = nc.dram_tensor("b_col", [HID, 1], f32, kind="ExternalInput")  # ln_b
    ident = nc.dram_tensor("ident", [HID, HID], DT, kind="ExternalInput")
    fixf = nc.dram_tensor("fixf", [1, K], f32, kind="ExternalInput")      # 12/count head
    fixl = nc.dram_tensor("fixl", [1, K], f32, kind="ExternalInput")      # 12/count tail
    # band-select matrix: hot column = 1/128 (stats row packing)
    selb = nc.dram_tensor("selb", [HID, 2 * 2 * NCH - 1], DT, kind="ExternalInput")
    outT = nc.dram_tensor("outT", [HID, N], DT, kind="ExternalOutput")

    Silu = mybir.ActivationFunctionType.Silu
    Sqrt = mybir.ActivationFunctionType.Sqrt

    with tile.TileContext(nc) as tc:
        with (
            tc.tile_pool(name="singles", bufs=1) as singles,
            tc.tile_pool(name="big", bufs=1) as big,
            tc.tile_pool(name="work", bufs=1) as work,
            tc.tile_pool(name="zpool", bufs=3) as zpool,
            tc.tile_pool(name="opool", bufs=3) as opool,
            tc.tile_pool(name="dramp", bufs=1, space="DRAM") as dramp,
            tc.tile_pool(name="psA", bufs=1, space="PSUM") as psA,
            tc.tile_pool(name="psB", bufs=1, space="PSUM") as psB,
            tc.tile_pool(name="psS", bufs=1, space="PSUM") as psS,
        ):
            # ---- constants into SBUF (spread across DMA queues) ----
            sW1a = singles.tile([HID, HID], DT)
            sW1b = singles.tile([HID, HID], DT)
            sW2s = singles.tile([HID, HID], DT)
            sU1a = singles.tile([HID, HID], DT)
            sU1b = singles.tile([HID, HID], DT)
            sU2 = singles.tile([HID, HID], DT)
            sIdent = singles.tile([HID, HID], DT)
            sw1c = singles.tile([1, HID], DT)
            sw1cn = singles.tile([1, HID], DT)
            sb2u = singles.tile([1, HID], DT)
            sb1 = singles.tile([HID, 1], f32)
            sbu = singles.tile([HID, 1], f32)
            sg = singles.tile([HID, 1], f32)
            sb = singles.tile([HID, 1], f32)
            ssel = singles.tile([HID, 2 * 2 * NCH - 1], DT)
            # phase-a-critical weights first on the scalar/gpsimd queues; the
            # sync queue starts with the first h/coord chunks (issued in
            # load_chunk below, which the scheduler hoists ahead of these).
            qs = [nc.scalar, nc.gpsimd, nc.sync]
            loads = [(sW1b, W1b), (sw1c, w1c), (sW1a, W1a), (sw1cn, w1cn),
                     (sb1, b1c), (sW2s, W2s), (sU1a, U1a), (sU1b, U1b),
                     (sU2, U2), (sIdent, ident), (sb2u, b2u), (sbu, buc),
                     (sg, g_col), (sb, b_col), (ssel, selb)]
            for i, (sbuf_t, dr) in enumerate(loads):
                qs[i % 2].dma_start(out=sbuf_t, in_=dr[:, :])
            # broadcast [1,6] -> [128,6] fix tiles
            sfixf = singles.tile([HID, K], f32)
            sfixl = singles.tile([HID, K], f32)

            def bcast_rows(a, p=HID):
                return bass.AP(tensor=a.tensor, offset=a.offset,
                               ap=[[0, p]] + list(a.ap[1:]))

            nc.gpsimd.dma_start(out=sfixf, in_=bcast_rows(fixf[0:1, :]))
            nc.gpsimd.dma_start(out=sfixl, in_=bcast_rows(fixl[0:1, :]))
            sones_row = singles.tile([1, T], DT)
            nc.vector.memset(sones_row, 1.0)
            seps = singles.tile([NCH, 1], f32)
            nc.vector.memset(seps, float(EPS))

            # ---- big persistent buffers ----
            D_A = big.tile([HID, NCOL], DT)      # token j at col OFF0 + j
            D_B = big.tile([HID, NCOL], DT)      # token j at col OFF0 + 1 + j
            x_full = big.tile([HID, N], DT)
            # zero halo columns of D so boundary silu stays finite
            nc.vector.memset(D_A[:, 0:OFF0], 0.0)
            nc.vector.memset(D_A[:, OFF0 + N:NCOL], 0.0)
            nc.vector.memset(D_B[:, 0:OFF0 + 1], 0.0)
            nc.vector.memset(D_B[:, OFF0 + 1 + N:NCOL], 0.0)

            # LN stats: rows [0:32] = E[x]/chunk, [32:64] = E[x^2]/chunk
            st_ps = psS.tile([2 * NCH, T], f32)

            hts = {}
            crd = {}

            def load_chunk(c):
                ht = work.tile([HID, T], DT, tag="ht", bufs=6)
                nc.sync.dma_start(out=ht, in_=hT[:, c * T:(c + 1) * T])
                co = work.tile([1, T], DT, tag="co", bufs=6)
                nc.sync.dma_start(out=co, in_=coordR[:, c * T:(c + 1) * T])
                hts[c] = ht
                crd[c] = co

            def phase_a(c):
                # D chunk = W1b.T @ h  +  w1c x coord   (PSUM accumulate)
                d_ps = psA.tile([HID, T], f32, tag="de", bufs=2)
                nc.tensor.matmul(d_ps, sW1b, hts[c], start=True, stop=False)
                nc.tensor.matmul(d_ps, sw1c, crd[c], start=False, stop=True)
                col = OFF0 + c * T
                nc.vector.tensor_copy(D_A[:, col:col + T], d_ps)
                nc.gpsimd.tensor_copy(
                    out=D_B[:, col + 1:col + 1 + T], in_=D_A[:, col:col + T])

            def seg_in1(tile_ap, col, nseg):
                # [128, nseg, T] AP over D with outer column-stride 2
                s = tile_ap[:, col:col + T]
                return bass.AP(tensor=s.tensor, offset=s.offset,
                               ap=[s.ap[0], [2, nseg], [1, T]])

            def phase_b(t):
                ht, co = hts[t], crd[t]
                # E chunk = W1a.T @ h - w1c x coord
                e_ps = psA.tile([HID, T], f32, tag="de", bufs=2)
                nc.tensor.matmul(e_ps, sW1a, ht, start=True, stop=False)
                nc.tensor.matmul(e_ps, sw1cn, co, start=False, stop=True)
                e_sb = work.tile([HID, T], DT, tag="esb", bufs=3)
                nc.vector.tensor_copy(e_sb, e_ps)

                # Z: 12 segments of E + shifted D, 3 stride-2 groups
                z = zpool.tile([HID, 12 * T], DT, tag="z")
                zv = z.rearrange("p (s t) -> p s t", t=T)

                def e_bc(nseg):
                    return bass.AP(tensor=e_sb.tensor, offset=e_sb.offset,
                                   ap=[e_sb.ap[0], [0, nseg], [1, T]])
                base = t * T
                groups = [
                    (D_A, OFF0 + base + NEG_EVEN[0], 0, 3),
                    (D_B, OFF0 + 1 + base + ODD[0], 3, 6),
                    (D_A, OFF0 + base + POS_EVEN[0], 9, 3),
                ]
                for dbuf, col, s0, nseg in groups:
                    nc.vector.tensor_tensor(
                        out=zv[:, s0:s0 + nseg, :],
                        in0=e_bc(nseg), in1=seg_in1(dbuf, col, nseg),
                        op=mybir.AluOpType.add)

                # silu over all 12 segments at once (bias = msg_b1)
                nc.scalar.activation(z, z, Silu, bias=sb1, scale=1.0)

                # zero invalid boundary columns (torn edges of the sequence)
                if t == 0:
                    for s, d in enumerate(SEG_ORDER):
                        if d < 0:
                            nc.vector.memset(zv[:, s, 0:-d], 0.0)
                if t == NCH - 1:
                    for s, d in enumerate(SEG_ORDER):
                        if d > 0:
                            nc.vector.memset(zv[:, s, T - d:T], 0.0)

                # agg_pre = sum_s silu(z_s) @ W2s   (PSUM accumulation)
                a_ps = psB.tile([HID, T], f32, tag="agg", bufs=2)
                for s in range(12):
                    nc.tensor.matmul(a_ps, sW2s, zv[:, s, :],
                                     start=(s == 0), stop=(s == 11))
                agg = work.tile([HID, T], DT, tag="agg_sb", bufs=3)
                nc.vector.tensor_copy(agg, a_ps)
                if t == 0:
                    nc.vector.tensor_tensor(out=agg[:, 0:K], in0=a_ps[:, 0:K],
                                            in1=sfixf, op=mybir.AluOpType.mult)
                if t == NCH - 1:
                    nc.vector.tensor_tensor(out=agg[:, T - K:T],
                                            in0=a_ps[:, T - K:T],
                                            in1=sfixl, op=mybir.AluOpType.mult)

                # update MLP
                u_ps = psA.tile([HID, T], f32, tag="upd", bufs=2)
                nc.tensor.matmul(u_ps, sU1a, ht, start=True, stop=False)
                nc.tensor.matmul(u_ps, sU1b, agg, start=False, stop=True)
                s2 = work.tile([HID, T], DT, tag="s2", bufs=3)
                nc.scalar.activation(s2, u_ps, Silu, bias=sbu, scale=1.0)

                # x = h + silu@U2 + b2u  (all accumulated in PSUM)
                x_ps = psA.tile([HID, T], f32, tag="xps", bufs=1)
                nc.tensor.matmul(x_ps, sU2, s2, start=True, stop=False)
                nc.tensor.matmul(x_ps, sb2u, sones_row, start=False, stop=False)
                nc.tensor.matmul(x_ps, sIdent, ht, start=False, stop=True)
                x_sb = x_full[:, base:base + T]
                nc.vector.tensor_copy(x_sb, x_ps)
                x2 = work.tile([HID, T], DT, tag="x2", bufs=2)
                nc.vector.tensor_tensor(out=x2, in0=x_sb, in1=x_sb,
                                        op=mybir.AluOpType.mult)
                # LN stats rows: band-select lhsT packs E[x] into psum row t
                # and E[x^2] into row NCH+t of one accumulating [64,T] bank
                hot = 2 * NCH - 1
                nc.tensor.matmul(st_ps[:, :], ssel[:, hot - t:hot - t + 2 * NCH],
                                 x_sb, start=(t == 0), stop=False)
                nc.tensor.matmul(st_ps[:, :],
                                 ssel[:, hot - NCH - t:hot - t + NCH],
                                 x2, start=False, stop=(t == NCH - 1))

            # ---------------- pass 1 ----------------
            load_chunk(0)
            for c in range(NCH + 1):
                if c < NCH:
                    if c + 1 < NCH:
                        load_chunk(c + 1)
                    phase_a(c)
                if c >= 1:
                    phase_b(c - 1)

            # ---------------- LN stats math ----------------
            # mr16 packs [mu | rstd] rows side by side -> one DRAM round trip
            # and one broadcast DMA per chunk in pass 2.
            mr16 = work.tile([NCH, 2 * T], DT, tag="mr16")
            nc.vector.tensor_copy(mr16[:, 0:T], st_ps[0:NCH, :])
            ex32 = work.tile([NCH, T], f32, tag="ex")
            nc.vector.tensor_copy(ex32, st_ps[0:NCH, :])
            musq = work.tile([NCH, T], f32, tag="musq")
            nc.vector.tensor_tensor(out=musq, in0=ex32, in1=ex32,
                                    op=mybir.AluOpType.mult)
            var = work.tile([NCH, T], f32, tag="var")
            nc.vector.tensor_tensor(out=var, in0=st_ps[NCH:2 * NCH, :], in1=musq,
                                    op=mybir.AluOpType.subtract)
            sd32 = work.tile([NCH, T], f32, tag="sd32")
            nc.scalar.activation(sd32, var, Sqrt, bias=seps, scale=1.0)
            r32 = work.tile([NCH, T], f32, tag="r32")
            with nc.allow_low_precision(reason="rstd rows feed fp16 normalize"):
                nc.vector.reciprocal_approx_fast(out=r32, in_=sd32)
            nc.vector.tensor_copy(mr16[:, T:2 * T], r32)
            mr_dr = dramp.tile([NCH, 2 * T], DT)
            nc.sync.dma_start(out=mr_dr, in_=mr16)

            # ---------------- pass 2: normalize (no matmuls) ----------------
            Ident = mybir.ActivationFunctionType.Identity
            for t in range(NCH):
                base = t * T
                mr_bc = opool.tile([HID, 2 * T], DT, tag="mr_bc", bufs=6)
                qa = nc.scalar if t % 2 == 0 else nc.gpsimd
                qa.dma_start(out=mr_bc, in_=bcast_rows(mr_dr[t:t + 1, :]))
                t0 = work.tile([HID, T], DT, tag="t0", bufs=4)
                nc.vector.tensor_tensor(out=t0, in0=x_full[:, base:base + T],
                                        in1=mr_bc[:, 0:T],
                                        op=mybir.AluOpType.subtract)
                t1 = work.tile([HID, T], DT, tag="t1", bufs=4)
                nc.vector.tensor_tensor(out=t1, in0=t0, in1=mr_bc[:, T:2 * T],
                                        op=mybir.AluOpType.mult)
                o = opool.tile([HID, T], DT, tag="o", bufs=6)
                nc.scalar.activation(o, t1, Ident, bias=sb, scale=sg)
                nc.sync.dma_start(out=outT[:, base:base + T], in_=o)

    nc.compile()
    return nc


def _get_compiled(dt_name):
    global _compiled
    if _compiled is None:
        from concourse import mybir
        dt = {"bf16": mybir.dt.bfloat16, "fp16": mybir.dt.float16, "fp32": mybir.dt.float32}[dt_name]
        _compiled = _build_bass(dt)
    return _compiled


DT_NAME = "fp16"


def _sel_band(act_np):
    hot = 2 * NCH - 1
    sel = np.zeros((HID, 2 * 2 * NCH - 1), dtype=np.float32)
    sel[:, hot] = 1.0 / HID
    return sel.astype(act_np)


def kernel(**inputs):
    from concourse.bass_utils import run_bass_kernel_spmd

    h = np.asarray(inputs["h"], dtype=np.float32)
    coord = np.asarray(inputs["coord"], dtype=np.float32)
    msg_w1 = np.asarray(inputs["msg_w1"], dtype=np.float32)
    msg_b1 = np.asarray(inputs["msg_b1"], dtype=np.float32)
    msg_w2 = np.asarray(inputs["msg_w2"], dtype=np.float32)
    msg_b2 = np.asarray(inputs["msg_b2"], dtype=np.float32)
    upd_w1 = np.asarray(inputs["upd_w1"], dtype=np.float32)
    upd_b1 = np.asarray(inputs["upd_b1"], dtype=np.float32)
    upd_w2 = np.asarray(inputs["upd_w2"], dtype=np.float32)
    upd_b2 = np.asarray(inputs["upd_b2"], dtype=np.float32)
    ln_g = np.asarray(inputs["ln_g"], dtype=np.float32)
    ln_b = np.asarray(inputs["ln_b"], dtype=np.float32)

    import ml_dtypes
    act_np = {"bf16": ml_dtypes.bfloat16, "fp16": np.float16, "fp32": np.float32}[DT_NAME]

    W1a = msg_w1[:HID]
    W1b = msg_w1[HID:2 * HID]
    w1c = msg_w1[2 * HID]
    bias_u = upd_b1 + msg_b2 @ upd_w1[HID:2 * HID]
    W2s = msg_w2 / (2.0 * K)

    idx = np.arange(N)
    count = (np.minimum(idx, K) + np.minimum(N - 1 - idx, K)).astype(np.float32)
    fix = (2.0 * K) / count
    fixf = fix[:K].reshape(1, K).astype(np.float32)
    fixl = fix[N - K:].reshape(1, K).astype(np.float32)

    const = {
        "W1a": np.ascontiguousarray(W1a, dtype=act_np),
        "W1b": np.ascontiguousarray(W1b, dtype=act_np),
        "w1c": np.ascontiguousarray(w1c.reshape(1, HID), dtype=act_np),
        "w1cn": np.ascontiguousarray(-w1c.reshape(1, HID), dtype=act_np),
        "W2s": np.ascontiguousarray(W2s, dtype=act_np),
        "U1a": np.ascontiguousarray(upd_w1[:HID], dtype=act_np),
        "U1b": np.ascontiguousarray(upd_w1[HID:], dtype=act_np),
        "U2": np.ascontiguousarray(upd_w2, dtype=act_np),
        "b1c": np.ascontiguousarray(msg_b1.reshape(HID, 1), dtype=np.float32),
        "buc": np.ascontiguousarray(bias_u.reshape(HID, 1), dtype=np.float32),
        "b2u": np.ascontiguousarray(upd_b2.reshape(1, HID), dtype=act_np),
        "g_col": np.ascontiguousarray(ln_g.reshape(HID, 1), dtype=np.float32),
        "b_col": np.ascontiguousarray(ln_b.reshape(HID, 1), dtype=np.float32),
        "ident": np.ascontiguousarray(np.eye(HID), dtype=act_np),
        "fixf": fixf,
        "fixl": fixl,
        "selb": _sel_band(act_np),
    }

    in_maps = []
    for b in range(B):
        m = dict(const)
        m["hT"] = np.ascontiguousarray(h[b].T, dtype=act_np)
        m["coordR"] = np.ascontiguousarray(coord[b].reshape(1, N), dtype=act_np)
        in_maps.append(m)

    nc = _get_compiled(DT_NAME)
    res = run_bass_kernel_spmd(nc, in_maps, core_ids=list(range(B)))
    global LAST_RESULTS
    LAST_RESULTS = res
    out = np.stack([np.asarray(res.results[b]["outT"], dtype=np.float32).T
                    for b in range(B)])
    return np.ascontiguousarray(out)
